# revision 17
# baseline (speedup 1.0000x reference)
"""Trainium2 Bass kernel for nn_CustomAttn: qkv proj + flat-axis qk-RMSnorm +
RoPE + causal attention + out proj, tensor-parallel over heads (Megatron-style)
with data-parallel batch, on 8 NeuronCores.

Mesh: core c -> batch b = c // 4, head-group hg = c % 4 (heads hg*4 .. hg*4+3).
Groups [[0,1,2,3],[4,5,6,7]]: AllReduce for the qk-norm sum-of-squares (the
norm is over the flat 2048-dim axis, i.e. across all 16 heads), ReduceScatter
(by token) for the output-projection partial sums.  Core c returns tokens
[(c%4)*512 : (c%4+1)*512] of batch c//4.
"""

import sys

for p in ("/opt/trn_rl_repo",):
    if p not in sys.path:
        sys.path.insert(0, p)

import numpy as np
import ml_dtypes
from contextlib import ExitStack

import concourse.bass as bass
import concourse.bacc as bacc
from concourse.tile import TileContext
from concourse import mybir
from concourse.bass_utils import run_bass_kernel_spmd

BF16 = mybir.dt.bfloat16
F32 = mybir.dt.float32
NPBF16 = ml_dtypes.bfloat16

B, S, HID = 2, 2048, 2048
NH, HD = 16, 128
EPS = 1e-5
ROPE_BASE = 10000.0

NCORES = 8
TPG = 4                    # tensor-parallel group size
NHL = NH // TPG            # 4 local heads
DL = NHL * HD              # 512 local q/k/v dims
KT = HID // 128            # 16 contraction chunks
NT = S // 512              # 4 token tiles of 512
TB = S // 128              # 16 token blocks of 128
GROUPS = [[0, 1, 2, 3], [4, 5, 6, 7]]
SCALE = 1.0 / float(np.sqrt(HD))

LAST_EXEC_NS = None
_CACHED_NC = None


def build_nc():
    nc = bacc.Bacc(num_devices=NCORES)

    xT = nc.declare_dram_parameter("xT", [HID, S], BF16, isOutput=False)
    w_inT = nc.declare_dram_parameter("w_inT", [HID, 3 * DL], BF16, isOutput=False)
    w_outT = nc.declare_dram_parameter("w_outT", [DL, HID], BF16, isOutput=False)
    qn = nc.declare_dram_parameter("qn", [128, 4], F32, isOutput=False)
    kn = nc.declare_dram_parameter("kn", [128, 4], F32, isOutput=False)
    c128 = nc.declare_dram_parameter("c128", [128, S], BF16, isOutput=False)
    s128 = nc.declare_dram_parameter("s128", [128, S], BF16, isOutput=False)
    maskT = nc.declare_dram_parameter("maskT", [128, 128], BF16, isOutput=False)
    out = nc.declare_dram_parameter("out", [512, HID], F32, isOutput=True)

    cc_in = nc.dram_tensor("cc_in", [2, S], F32)
    cc_out = nc.dram_tensor("cc_out", [2, S], F32)
    op_buf = nc.dram_tensor("op_buf", [4, S, 512], BF16)
    rs_out = nc.dram_tensor("rs_out", [4, 512, 512], BF16)

    with TileContext(nc) as tc, ExitStack() as ctx:
        consts = ctx.enter_context(tc.tile_pool(name="consts", bufs=1))
        weights = ctx.enter_context(tc.tile_pool(name="weights", bufs=1))
        persist = ctx.enter_context(tc.tile_pool(name="persist", bufs=1))
        xpool = ctx.enter_context(tc.tile_pool(name="xpool", bufs=1))
        sqp = ctx.enter_context(tc.tile_pool(name="sqp", bufs=2))
        mmp = ctx.enter_context(tc.tile_pool(name="mmp", bufs=3, space="PSUM"))
        accp = ctx.enter_context(tc.tile_pool(name="accp", bufs=2, space="PSUM"))
        smallp = ctx.enter_context(tc.tile_pool(name="smallp", bufs=2, space="PSUM"))
        ropet = ctx.enter_context(tc.tile_pool(name="ropet", bufs=2))
        rqp = ctx.enter_context(tc.tile_pool(name="rqp", bufs=2))
        expp = ctx.enter_context(tc.tile_pool(name="expp", bufs=4))
        wop = ctx.enter_context(tc.tile_pool(name="wop", bufs=2))
        outp = ctx.enter_context(tc.tile_pool(name="outp", bufs=1))

        # --- constants ---
        ones_col = consts.tile([128, 1], F32)          # lhsT for partition-sum
        nc.vector.memset(ones_col, 1.0)
        ones_row = consts.tile([1, 128], F32)          # lhsT for bcast over partitions
        nc.vector.memset(ones_row, 1.0)
        qn_t = consts.tile([128, 4], F32)
        nc.sync.dma_start(out=qn_t, in_=qn[:, :])
        kn_t = consts.tile([128, 4], F32)
        nc.sync.dma_start(out=kn_t, in_=kn[:, :])
        mask_t = consts.tile([128, 128], BF16)
        nc.sync.dma_start(out=mask_t, in_=maskT[:, :])
        ones_col_b = consts.tile([128, 1], BF16)       # bf16 ones for denom
        nc.vector.memset(ones_col_b, 1.0)
        zeros_b = consts.tile([128, 1], F32)           # explicit bias for Exp
        nc.vector.memset(zeros_b, 0.0)
        eps_b = consts.tile([1, 1], F32)               # explicit bias for Sqrt
        nc.vector.memset(eps_b, EPS)

        # --- resident weights ---
        w_tiles = []
        for k in range(KT):
            wt = weights.tile([128, 3 * DL], BF16, tag=f"w{k}", name=f"w{k}")
            nc.sync.dma_start(out=wt, in_=w_inT[k * 128:(k + 1) * 128, :])
            w_tiles.append(wt)

        # q/k staging (rope is applied in place later): m 0..3 = q chunks,
        # m 4..7 = k chunks, each [128 dims, S tokens] bf16
        qk_tiles = [persist.tile([128, S], BF16, tag=f"qk{m}", name=f"qk{m}") for m in range(8)]
        # v in token-major layout: [128 tokens, 512 vdims] per token block
        v_tiles = [persist.tile([128, DL], BF16, tag=f"v{tb}", name=f"v{tb}") for tb in range(TB)]

        # ---------- phase 1: qkv projection + sum-of-squares ----------
        for n in range(NT):
            xt = []
            for k in range(KT):
                t = xpool.tile([128, 512], BF16, tag=f"x{k}", name=f"x{k}")
                nc.sync.dma_start(
                    out=t, in_=xT[k * 128:(k + 1) * 128, n * 512:(n + 1) * 512])
                xt.append(t)

            for ti, base in ((0, 0), (1, 4)):       # q then k chunks
                ssq_ps = smallp.tile([1, 512], F32, tag="ssq")
                for mi in range(4):
                    m = base + mi
                    pq = mmp.tile([128, 512], F32, tag="mm")
                    for k in range(KT):
                        nc.tensor.matmul(
                            pq, w_tiles[k][:, m * 128:(m + 1) * 128], xt[k],
                            start=(k == 0), stop=(k == KT - 1))
                    sq = sqp.tile([128, 512], F32, tag="sq")
                    nc.scalar.square(sq, pq)
                    nc.tensor.matmul(ssq_ps, ones_col, sq,
                                     start=(mi == 0), stop=(mi == 3))
                    ncol = (qn_t if ti == 0 else kn_t)[:, mi:mi + 1]
                    nc.scalar.mul(qk_tiles[m][:, n * 512:(n + 1) * 512], pq, ncol)
                ssq_s = sqp.tile([1, 512], F32, tag="invd", name="ssq_s", bufs=2)
                nc.scalar.copy(ssq_s, ssq_ps)
                nc.sync.dma_start(
                    out=cc_in[ti:ti + 1, n * 512:(n + 1) * 512], in_=ssq_s)

            for tbl in range(4):                    # v projection, token-major
                tb = n * 4 + tbl
                pv = mmp.tile([128, 512], F32, tag="mm")
                for k in range(KT):
                    nc.tensor.matmul(
                        pv, xt[k][:, tbl * 128:(tbl + 1) * 128],
                        w_tiles[k][:, 2 * DL:3 * DL],
                        start=(k == 0), stop=(k == KT - 1))
                nc.vector.tensor_copy(v_tiles[tb], pv)

        # ---------- phase 2: allreduce sumsq, inv_rms, rope tables ----------
        nc.gpsimd.collective_compute(
            "AllReduce", mybir.AluOpType.add, replica_groups=GROUPS,
            ins=[cc_in[:, :]], outs=[cc_out[:, :]])

        inv_t = []
        for t in range(2):
            tot = persist.tile([1, S], F32, tag=f"tot{t}", name=f"tot{t}")
            nc.sync.dma_start(out=tot, in_=cc_out[t:t + 1, :])
            nc.scalar.activation(tot, tot, mybir.ActivationFunctionType.Sqrt,
                                 bias=eps_b, scale=1.0 / (NH * HD))
            nc.vector.reciprocal(tot, tot)
            inv_t.append(tot)

        # cos/sin premultiplied by inv_rms, bcast over partitions via ones
        ci = [persist.tile([128, S], BF16, tag=f"ci{t}", name=f"ci{t}") for t in range(2)]
        si = [persist.tile([128, S], BF16, tag=f"si{t}", name=f"si{t}") for t in range(2)]
        for t in range(2):
            for j in range(NT):
                sl = slice(j * 512, (j + 1) * 512)
                bc = mmp.tile([128, 512], F32, tag="mm")
                nc.tensor.matmul(bc, ones_row, inv_t[t][:, sl],
                                 start=True, stop=True)
                cs = ropet.tile([128, 512], BF16, tag="rt", name="cs")
                nc.sync.dma_start(out=cs, in_=c128[:, sl])
                nc.vector.tensor_mul(ci[t][:, sl], cs, bc)
                ss = ropet.tile([128, 512], BF16, tag="rt2", name="ss")
                nc.sync.dma_start(out=ss, in_=s128[:, sl])
                nc.vector.tensor_mul(si[t][:, sl], ss, bc)

        # ---------- phase 3: rope (in place on qk staging) ----------
        for m in range(8):
            t = 0 if m < 4 else 1
            qk = qk_tiles[m]
            rq = rqp.tile([128, S], BF16, tag="rq", name="rq")
            nc.sync.dma_start(out=rq[0:64, :], in_=qk[64:128, :])
            nc.sync.dma_start(out=rq[64:128, :], in_=qk[0:64, :])
            nc.vector.tensor_mul(qk, qk, ci[t])
            nc.vector.tensor_mul(rq, rq, si[t])
            nc.vector.tensor_add(qk, qk, rq)

        # ---------- phase 4: attention ----------
        w_out_tiles = []
        for h in range(NHL):
            wt = persist.tile([128, HID], BF16, tag=f"wo{h}", name=f"wot{h}")
            nc.sync.dma_start(out=wt, in_=w_outT[h * 128:(h + 1) * 128, :])
            w_out_tiles.append(wt)

        attnT = [persist.tile([128, S], BF16, tag=f"at{h}", name=f"at{h}") for h in range(NHL)]

        for h in range(NHL):
            kt_h, qt_h = qk_tiles[4 + h], qk_tiles[h]
            for j in range(NT):
                pv_ps = accp.tile([128, 512], F32, tag="pv")
                den_ps = smallp.tile([1, 512], F32, tag="ssq")
                nb = 4 * j + 4
                for b in range(nb):
                    r = b - 4 * j
                    q_off = max(r, 0) * 128
                    w = 512 - q_off
                    s_ps = mmp.tile([128, 512], F32, tag="mm")
                    nc.tensor.matmul(
                        s_ps[:, :w], kt_h[:, b * 128:(b + 1) * 128],
                        qt_h[:, j * 512 + q_off:(j + 1) * 512],
                        start=True, stop=True)
                    ex = expp.tile([128, 512], BF16, tag="exp")
                    nc.scalar.activation(ex[:, :w], s_ps[:, :w],
                                         mybir.ActivationFunctionType.Exp,
                                         bias=zeros_b, scale=SCALE)
                    if r >= 0:
                        nc.vector.tensor_mul(ex[:, 0:128], ex[:, 0:128], mask_t)
                    nc.tensor.matmul(
                        pv_ps[:, q_off:512],
                        v_tiles[b][:, h * 128:(h + 1) * 128], ex[:, :w],
                        start=(b == 0), stop=(b == nb - 1))
                    nc.tensor.matmul(
                        den_ps[0:1, q_off:512], ones_col_b, ex[:, :w],
                        start=(b == 0), stop=(b == nb - 1))
                inv_d = sqp.tile([1, 512], F32, tag="invd", bufs=2)
                nc.vector.reciprocal(inv_d, den_ps)
                bc = mmp.tile([128, 512], F32, tag="mm")
                nc.tensor.matmul(bc, ones_row, inv_d, start=True, stop=True)
                bc_sb = sqp.tile([128, 512], F32, tag="sq", name="bc_sb")
                nc.scalar.copy(bc_sb, bc)
                nc.vector.tensor_mul(attnT[h][:, j * 512:(j + 1) * 512],
                                     pv_ps, bc_sb)

        # ---------- phase 5: output projection + reduce-scatter ----------
        for c in range(4):
            for tb in range(TB):
                po = mmp.tile([128, 512], F32, tag="mm")
                for h in range(NHL):
                    nc.tensor.matmul(
                        po, attnT[h][:, tb * 128:(tb + 1) * 128],
                        w_out_tiles[h][:, c * 512:(c + 1) * 512],
                        start=(h == 0), stop=(h == NHL - 1))
                ws = wop.tile([128, 512], BF16, tag="wo")
                nc.vector.tensor_copy(ws, po)
                nc.sync.dma_start(
                    out=op_buf[c, tb * 128:(tb + 1) * 128, :], in_=ws)
            nc.gpsimd.collective_compute(
                "ReduceScatter", mybir.AluOpType.add, replica_groups=GROUPS,
                ins=[op_buf[c]], outs=[rs_out[c]])

        # ---------- phase 6: gather shard, convert to f32, store ----------
        for c in range(4):
            for sb in range(4):
                t = outp.tile([128, 512], BF16, tag="fin_b")
                nc.sync.dma_start(
                    out=t, in_=rs_out[c, sb * 128:(sb + 1) * 128, :])
                t32 = outp.tile([128, 512], F32, tag="fin_f")
                nc.vector.tensor_copy(t32, t)
                nc.sync.dma_start(
                    out=out[sb * 128:(sb + 1) * 128, c * 512:(c + 1) * 512],
                    in_=t32)

    nc.finalize()
    return nc


def make_in_maps(x, w_in, w_out, q_norm_w, k_norm_w):
    x = np.asarray(x, np.float32)
    w_in = np.asarray(w_in, np.float32)
    w_out = np.asarray(w_out, np.float32)
    q_norm_w = np.asarray(q_norm_w, np.float32)
    k_norm_w = np.asarray(k_norm_w, np.float32)

    half = HD // 2
    inv_freq = 1.0 / (ROPE_BASE ** (np.arange(half, dtype=np.float32) / half))
    pos = np.arange(S, dtype=np.float32)
    ang = pos[:, None] * inv_freq[None, :]              # [S, 64]
    cos = np.cos(ang).T                                 # [64, S]
    sin = np.sin(ang).T
    c128 = np.concatenate([cos, cos], axis=0).astype(NPBF16)   # [128, S]
    s128 = np.concatenate([-sin, sin], axis=0).astype(NPBF16)  # rotate-half signs
    maskT = (np.arange(128)[:, None] <= np.arange(128)[None, :]).astype(NPBF16)

    in_maps = []
    for c in range(NCORES):
        b, hg = c // TPG, c % TPG
        rows = np.concatenate([
            w_in[hg * DL:(hg + 1) * DL],
            w_in[NH * HD + hg * DL:NH * HD + (hg + 1) * DL],
            w_in[2 * NH * HD + hg * DL:2 * NH * HD + (hg + 1) * DL],
        ], axis=0)                                      # [1536, HID]
        in_maps.append({
            "xT": np.ascontiguousarray(x[b].T).astype(NPBF16),
            "w_inT": np.ascontiguousarray(rows.T).astype(NPBF16),
            "w_outT": np.ascontiguousarray(
                w_out[:, hg * DL:(hg + 1) * DL].T).astype(NPBF16),
            "qn": np.ascontiguousarray(
                q_norm_w[hg * DL:(hg + 1) * DL].reshape(4, 128).T),
            "kn": np.ascontiguousarray(
                k_norm_w[hg * DL:(hg + 1) * DL].reshape(4, 128).T),
            "c128": c128, "s128": s128, "maskT": maskT,
        })
    return in_maps


def kernel(x, w_in, w_out, q_norm_w, k_norm_w, trace=False):
    global LAST_EXEC_NS, _CACHED_NC
    if _CACHED_NC is None:
        _CACHED_NC = build_nc()
    nc = _CACHED_NC
    in_maps = make_in_maps(x, w_in, w_out, q_norm_w, k_norm_w)
    res = run_bass_kernel_spmd(nc, in_maps, list(range(NCORES)), trace=trace)
    LAST_EXEC_NS = res.exec_time_ns
    outp = np.empty((B, S, HID), np.float32)
    for c in range(NCORES):
        b, t = c // TPG, c % TPG
        outp[b, t * 512:(t + 1) * 512, :] = np.asarray(
            res.results[c]["out"], np.float32)
    return outp
